# revision 26
# baseline (speedup 1.0000x reference)
"""CapsuleLayer (single routing iteration) Trainium2 kernel.

Math (per batch element b of x: (B=64, NU=32, IC=256, US=128) fp32):
  - torch-style reshape of x[b] to (IC, NU, US): row i of the flat
    (256, 4096) view is x[b].flat[i*4096:(i+1)*4096].
  - s[j]   = (1/256) * sum_i flat[i, j]          (j = n*128+u, 4096 outputs)
  - msq[n] = sum_u s[n,u]^2
  - out[n,u] = msq/(1+msq) * s[n,u]/(sqrt(msq)+1e-5)

Sharding: pure batch data-parallel over 8 NeuronCores (8 batches/core).

Per-core pipeline (memory-bound; ~32 MiB HBM reads per core):
  - Every (batch, ic-half) 2 MiB tile is split 50/50 by column across the
    two HWDGE rings (SP ring gets cols [0:2048], ACT ring gets cols
    [2048:4096]) so both rings always carry identical outstanding bytes
    and finish together -- the rings' drain rates are not reliably equal,
    and a ring that finishes early leaves a single-queue (~340 GB/s <
    ~430 GB/s fabric) tail.
  - Per-chunk float32r matmuls (lhsT = 128x1 column of 1/256) accumulate
    chunk k of both ic-halves into PSUM partition k. Per-partition
    accumulation groups mean PSUM partitions 0-3 depend only on the SP
    ring and 4-7 only on the ACT ring, so each half of the squash tail
    can start as soon as its own ring's data was consumed.
  - Squash is computed per partition-half: squares+sums on the (otherwise
    idle) Vector engine via tensor_tensor_reduce, a tiny Sqrt on Scalar
    (emitted with a 2-batch lookahead so the ACT ring's load issues are
    never blocked behind it), factor algebra + final multiply on Vector.
  - Stores ride SWDGE (gpsimd) mid-run and the empty HWDGE rings for the
    final batch.

float32r streams fp32 through the PE in one pass by rounding the operands
to ~12 mantissa bits (same 4-byte encoding, so the host feeds plain fp32
bytes); weights are powers of two (exact), so output rel-err is ~1e-4
instead of fp32's ~1e-7, while PE time drops 4x vs the two-pass fp32 path.
"""

import numpy as np

import concourse.bass as bass
import concourse.bacc as bacc
import concourse.mybir as mybir
import concourse.tile as tile
from concourse.bass_utils import run_bass_kernel_spmd

B, NU, IC, US = 64, 32, 256, 128
N_CORES = 8
PB = B // N_CORES            # batches per core
F = NU * US                  # 4096 outputs per batch
HALVES = IC // 128           # 2 partition-halves of the ic axis
NCHUNK = 8                   # 512-col PSUM chunks per batch
HF = F // 2                  # 2048 cols per ring-half
NQ = 4                       # 128-col u-groups per PSUM partition
LOOKAHEAD = 2                # batches of load issue ahead of squash work

mm_dt = mybir.dt.float32r

# bisect flags
SQUASH_MODE = "dve"    # "dve": tensor_tensor_reduce; "act": Square+accum
STORE_LAST = "hwdge"   # "hwdge": last batch stores on SP/ACT; else SWDGE


def build_bass(pb=PB, squash_mode=None, store_last=None):
    PB = pb
    squash_mode = squash_mode or SQUASH_MODE
    store_last = store_last or STORE_LAST
    nc = bacc.Bacc("TRN2", target_bir_lowering=False, debug=False)

    # float32r shares the fp32 byte encoding (it is fp32 with the mantissa
    # rounded to ~12 bits by the PE), so the host feeds plain fp32 bytes.
    x = nc.dram_tensor("x", [PB, HALVES, 128, F], mm_dt,
                       kind="ExternalInput")
    w = nc.dram_tensor("w", [128, 5, 4], mm_dt, kind="ExternalInput")
    y = nc.dram_tensor("y", [PB, NCHUNK, 512], mybir.dt.float32,
                       kind="ExternalOutput")

    f32 = mybir.dt.float32

    with tile.TileContext(nc) as tc:
        with (
            tc.tile_pool(name="const", bufs=1) as const_pool,
            tc.tile_pool(name="acc", bufs=11) as acc_pool,
            tc.tile_pool(name="psum", bufs=8, space="PSUM") as psum_pool,
            tc.tile_pool(name="scratch", bufs=2) as scratch_pool,
            tc.tile_pool(name="stats", bufs=2) as stats_pool,
            tc.tile_pool(name="outp", bufs=3) as out_pool,
        ):
            # sel[:, m, j] = 1/256 iff j == m (m < 4): chunk k's 512 sums
            # land in PSUM partition k%4 of its half-region. Column m == 4
            # is all-zero -- used by "warmer" matmuls that accumulate +0.
            # Loaded via SWDGE so the HWDGE rings open with full-rate loads.
            sel = const_pool.tile([128, 5, 4], mm_dt)
            nc.gpsimd.dma_start(out=sel[:], in_=w[:])

            ps_tiles = [None] * PB

            # The PE HAM clock gate halves the PE clock (K=4/8) whenever the
            # PE has been idle recently; at half clock the fp32r matmuls
            # fall behind the DMA stream and the whole tail piles up. The
            # warmers are zero-weight matmuls that keep the PE busy through
            # the natural per-tile idle gaps so it stays at full clock.
            ND = 3

            def warmers(ps, t, n=ND):
                for d in range(n):
                    nc.tensor.matmul(
                        ps[d % 2][:, :],
                        sel[:, 4, :],
                        t[:, (d % NCHUNK) * 512:((d % NCHUNK) + 1) * 512],
                        start=False, stop=False,
                    )

            def load_and_mm(b, h):
                t = acc_pool.tile([128, F], mm_dt, tag="acc")
                split = 2 if (b == PB - 1 and h == HALVES - 1) else 1
                # Ring A (SP) carries cols [0:HF], ring B (ACT) the rest;
                # the last tile goes in 512 KiB pieces so the PE trails the
                # final bytes closely.
                for eng, c0 in ((nc.sync, 0), (nc.scalar, HF)):
                    step = HF // split
                    for c in range(c0, c0 + HF, step):
                        eng.dma_start(out=t[:, c:c + step],
                                      in_=x[b, h, :, c:c + step])
                ps = ps_tiles[b]
                if ps is None:
                    # Ring A's chunks (0-3) accumulate into psA, ring B's
                    # (4-7) into psB (separate PSUM banks, both base
                    # partition 0 as the PE requires), so each squash half
                    # depends on exactly one ring.
                    psA = psum_pool.tile([4, 512], f32, tag="psA", bufs=4)
                    psB = psum_pool.tile([4, 512], f32, tag="psB", bufs=4)
                    ps = (psA, psB)
                    ps_tiles[b] = ps
                last_tile = b == PB - 1 and h == HALVES - 1
                if h == 0:
                    for k in range(NCHUNK):
                        nc.tensor.matmul(
                            ps[k // 4][:, :],
                            sel[:, k % 4, :],
                            t[:, k * 512:(k + 1) * 512],
                            start=(k % 4 == 0), stop=False,
                        )
                    warmers(ps, t)
                else:
                    # Keep the two group-closing matmuls last so warmers
                    # still land inside open accumulation groups.
                    for k in (0, 1, 2, 4, 5, 6):
                        nc.tensor.matmul(
                            ps[k // 4][:, :],
                            sel[:, k % 4, :],
                            t[:, k * 512:(k + 1) * 512],
                            start=False, stop=False,
                        )
                    if not last_tile:
                        warmers(ps, t)
                    for k in (3, 7):
                        nc.tensor.matmul(
                            ps[k // 4][:, :],
                            sel[:, k % 4, :],
                            t[:, k * 512:(k + 1) * 512],
                            start=False, stop=True,
                        )

            def squash(b):
                last = b == PB - 1
                H = NCHUNK // 2

                # Per half: land s (or s^2) in SBUF and fold each 128-wide
                # u-group into msq[4*hf:4*hf+4] via DVE tensor_reduce. For
                # all but the last batch a DVE copy releases the PSUM bank
                # immediately so the PE is never gated on trailing squash
                # work; the last batch reads PSUM directly (shorter chain).
                msq = stats_pool.tile([H, HALVES * NQ], f32, tag="msq")
                srcs = []
                for hf in range(HALVES):
                    psl = ps_tiles[b][hf][:, :]
                    tg = str(hf)
                    sq = scratch_pool.tile([H, 512], f32, tag="sq" + tg)
                    src = psl
                    srcs.append(src)
                    nc.scalar.activation(
                        out=sq[:], in_=src,
                        func=mybir.ActivationFunctionType.Square)
                    nc.vector.tensor_reduce(
                        out=msq[:, hf * NQ:(hf + 1) * NQ],
                        in_=sq[:].rearrange("p (q u) -> p q u", q=NQ),
                        axis=mybir.AxisListType.X,
                        op=mybir.AluOpType.add)

                # Joint factor = msq / ((1 + msq) * (sqrt(msq) + 1e-5))
                mag = stats_pool.tile([H, HALVES * NQ], f32, tag="mag")
                nc.scalar.activation(out=mag[:], in_=msq[:],
                                     func=mybir.ActivationFunctionType.Sqrt)
                t2 = stats_pool.tile([H, HALVES * NQ], f32, tag="t2")
                nc.vector.tensor_scalar_add(t2[:], mag[:], 1e-5)
                den = stats_pool.tile([H, HALVES * NQ], f32, tag="den")
                nc.vector.scalar_tensor_tensor(
                    out=den[:], in0=msq[:], scalar=1.0, in1=t2[:],
                    op0=mybir.AluOpType.add, op1=mybir.AluOpType.mult)
                rec = stats_pool.tile([H, HALVES * NQ], f32, tag="rec")
                nc.vector.reciprocal(rec[:], den[:])
                fac = stats_pool.tile([H, HALVES * NQ], f32, tag="fac")
                nc.vector.tensor_mul(fac[:], msq[:], rec[:])

                for hf in range(HALVES):
                    tg = str(hf)
                    outt = out_pool.tile([H, 512], f32, tag="out" + tg)
                    fap = fac[:, hf * NQ:(hf + 1) * NQ]
                    fac_bcast = bass.AP(tensor=fap.tensor, offset=fap.offset,
                                        ap=[fap.ap[0], fap.ap[1], [0, 128]])
                    nc.vector.tensor_mul(
                        outt[:].rearrange("p (q u) -> p q u", q=NQ),
                        srcs[hf].rearrange("p (q u) -> p q u", q=NQ),
                        fac_bcast)
                    if last:
                        # HWDGE rings are empty by now; keep the critical
                        # final stores off the slower SWDGE path.
                        eng = nc.sync if hf == 0 else nc.scalar
                    else:
                        eng = nc.gpsimd
                    eng.dma_start(out=y[b, hf * H:(hf + 1) * H],
                                  in_=outt[:])

            for b in range(PB):
                for h in range(HALVES):
                    load_and_mm(b, h)
                if b >= LOOKAHEAD:
                    squash(b - LOOKAHEAD)
            for b in range(PB - LOOKAHEAD, PB):
                squash(b)

    nc.compile()
    return nc


_NC_CACHE = {}


def _get_nc(**kw):
    key = tuple(sorted(kw.items()))
    if key not in _NC_CACHE:
        _NC_CACHE[key] = build_bass(**kw)
    return _NC_CACHE[key]


def kernel(x, **run_kwargs):
    x = np.ascontiguousarray(np.asarray(x, dtype=np.float32))
    assert x.shape == (B, NU, IC, US), x.shape

    nc = _get_nc()
    xs = x.reshape(N_CORES, PB, HALVES, 128, F)
    w = np.zeros((128, 5, 4), dtype=np.float32)
    for m in range(4):
        w[:, m, m] = 1.0 / IC
    in_maps = [{"x": np.ascontiguousarray(xs[c]), "w": w}
               for c in range(N_CORES)]
    res = run_bass_kernel_spmd(nc, in_maps, core_ids=list(range(N_CORES)),
                               **run_kwargs)
    out = np.stack([r["y"] for r in res.results], axis=0)  # (8, PB, 8, 512)
    out = out.reshape(B, NU, US, 1)
    if run_kwargs:
        kernel.last_results = res
    return out


# revision 29
# speedup vs baseline: 1.0204x; 1.0204x over previous
"""CapsuleLayer (single routing iteration) Trainium2 kernel.

Math (per batch element b of x: (B=64, NU=32, IC=256, US=128) fp32):
  - torch-style reshape of x[b] to (IC, NU, US): row i of the flat
    (256, 4096) view is x[b].flat[i*4096:(i+1)*4096].
  - s[j]   = (1/256) * sum_i flat[i, j]          (j = n*128+u, 4096 outputs)
  - msq[n] = sum_u s[n,u]^2
  - out[n,u] = msq/(1+msq) * s[n,u]/(sqrt(msq)+1e-5)

Sharding: pure batch data-parallel over 8 NeuronCores (8 batches/core).

Per-core pipeline (memory-bound; ~32 MiB HBM reads per core):
  - Every (batch, ic-half) 2 MiB tile is split 50/50 by column across the
    two HWDGE rings (SP ring gets cols [0:2048], ACT ring gets cols
    [2048:4096]) so both rings always carry identical outstanding bytes
    and finish together -- the rings' drain rates are not reliably equal,
    and a ring that finishes early leaves a single-queue (~340 GB/s <
    ~430 GB/s fabric) tail.
  - Per-chunk float32r matmuls (lhsT = 128x1 column of 1/256) accumulate
    chunk k of both ic-halves into PSUM partition k. Per-partition
    accumulation groups mean PSUM partitions 0-3 depend only on the SP
    ring and 4-7 only on the ACT ring, so each half of the squash tail
    can start as soon as its own ring's data was consumed.
  - Squash is computed per partition-half: squares+sums on the (otherwise
    idle) Vector engine via tensor_tensor_reduce, a tiny Sqrt on Scalar
    (emitted with a 2-batch lookahead so the ACT ring's load issues are
    never blocked behind it), factor algebra + final multiply on Vector.
  - Stores ride SWDGE (gpsimd) mid-run and the empty HWDGE rings for the
    final batch.

float32r streams fp32 through the PE in one pass by rounding the operands
to ~12 mantissa bits (same 4-byte encoding, so the host feeds plain fp32
bytes); weights are powers of two (exact), so output rel-err is ~1e-4
instead of fp32's ~1e-7, while PE time drops 4x vs the two-pass fp32 path.
"""

import numpy as np

import concourse.bass as bass
import concourse.bacc as bacc
import concourse.mybir as mybir
import concourse.tile as tile
from concourse.bass_utils import run_bass_kernel_spmd

B, NU, IC, US = 64, 32, 256, 128
N_CORES = 8
PB = B // N_CORES            # batches per core
F = NU * US                  # 4096 outputs per batch
HALVES = IC // 128           # 2 partition-halves of the ic axis
NCHUNK = 8                   # 512-col PSUM chunks per batch
HF = F // 2                  # 2048 cols per ring-half
NQ = 4                       # 128-col u-groups per PSUM partition
LOOKAHEAD = 2                # batches of load issue ahead of squash work

mm_dt = mybir.dt.float32r

# bisect flags
SQUASH_MODE = "dve"    # "dve": tensor_tensor_reduce; "act": Square+accum
STORE_LAST = "hwdge"   # "hwdge": last batch stores on SP/ACT; else SWDGE


def build_bass(pb=PB, squash_mode=None, store_last=None):
    PB = pb
    squash_mode = squash_mode or SQUASH_MODE
    store_last = store_last or STORE_LAST
    nc = bacc.Bacc("TRN2", target_bir_lowering=False, debug=False)

    # float32r shares the fp32 byte encoding (it is fp32 with the mantissa
    # rounded to ~12 bits by the PE), so the host feeds plain fp32 bytes.
    x = nc.dram_tensor("x", [PB, HALVES, 128, F], mm_dt,
                       kind="ExternalInput")
    w = nc.dram_tensor("w", [128, 5, 4], mm_dt, kind="ExternalInput")
    y = nc.dram_tensor("y", [PB, NCHUNK, 512], mybir.dt.float32,
                       kind="ExternalOutput")

    f32 = mybir.dt.float32

    with tile.TileContext(nc) as tc:
        with (
            tc.tile_pool(name="const", bufs=1) as const_pool,
            tc.tile_pool(name="acc", bufs=11) as acc_pool,
            tc.tile_pool(name="psum", bufs=8, space="PSUM") as psum_pool,
            tc.tile_pool(name="scratch", bufs=2) as scratch_pool,
            tc.tile_pool(name="stats", bufs=2) as stats_pool,
            tc.tile_pool(name="outp", bufs=3) as out_pool,
        ):
            # sel[:, m, j] = 1/256 iff j == m (m < 4): chunk k's 512 sums
            # land in PSUM partition k%4 of its half-region. Column m == 4
            # is all-zero -- used by "warmer" matmuls that accumulate +0.
            # Loaded via SWDGE so the HWDGE rings open with full-rate loads.
            sel = const_pool.tile([128, 5, 4], mm_dt)
            nc.gpsimd.dma_start(out=sel[:], in_=w[:])

            ps_tiles = [None] * PB

            def load_and_mm(b, h):
                t = acc_pool.tile([128, F], mm_dt, tag="acc")
                split = 2 if (b == PB - 1 and h == HALVES - 1) else 1
                # Ring A (SP) carries cols [0:HF], ring B (ACT) the rest;
                # the last tile goes in 512 KiB pieces so the PE trails the
                # final bytes closely.
                for eng, c0 in ((nc.sync, 0), (nc.scalar, HF)):
                    step = HF // split
                    for c in range(c0, c0 + HF, step):
                        eng.dma_start(out=t[:, c:c + step],
                                      in_=x[b, h, :, c:c + step])
                ps = ps_tiles[b]
                if ps is None:
                    # Ring A's chunks (0-3) accumulate into psA, ring B's
                    # (4-7) into psB (separate PSUM banks, both base
                    # partition 0 as the PE requires), so each squash half
                    # depends on exactly one ring.
                    psA = psum_pool.tile([4, 512], f32, tag="psA", bufs=4)
                    psB = psum_pool.tile([4, 512], f32, tag="psB", bufs=4)
                    ps = (psA, psB)
                    ps_tiles[b] = ps
                for k in range(NCHUNK):
                    nc.tensor.matmul(
                        ps[k // 4][:, :],
                        sel[:, k % 4, :],
                        t[:, k * 512:(k + 1) * 512],
                        start=(h == 0 and k % 4 == 0),
                        stop=(h == HALVES - 1 and k % 4 == 3),
                    )

            def squash(b):
                last = b == PB - 1
                H = NCHUNK // 2

                # Per half: land s (or s^2) in SBUF and fold each 128-wide
                # u-group into msq[4*hf:4*hf+4] via DVE tensor_reduce. For
                # all but the last batch a DVE copy releases the PSUM bank
                # immediately so the PE is never gated on trailing squash
                # work; the last batch reads PSUM directly (shorter chain).
                msq = stats_pool.tile([H, HALVES * NQ], f32, tag="msq")
                srcs = []
                for hf in range(HALVES):
                    psl = ps_tiles[b][hf][:, :]
                    tg = str(hf)
                    sq = scratch_pool.tile([H, 512], f32, tag="sq" + tg)
                    src = psl
                    srcs.append(src)
                    nc.scalar.activation(
                        out=sq[:], in_=src,
                        func=mybir.ActivationFunctionType.Square)
                    nc.vector.tensor_reduce(
                        out=msq[:, hf * NQ:(hf + 1) * NQ],
                        in_=sq[:].rearrange("p (q u) -> p q u", q=NQ),
                        axis=mybir.AxisListType.X,
                        op=mybir.AluOpType.add)

                # Joint factor = msq / ((1 + msq) * (sqrt(msq) + 1e-5))
                mag = stats_pool.tile([H, HALVES * NQ], f32, tag="mag")
                nc.scalar.activation(out=mag[:], in_=msq[:],
                                     func=mybir.ActivationFunctionType.Sqrt)
                t2 = stats_pool.tile([H, HALVES * NQ], f32, tag="t2")
                nc.vector.tensor_scalar_add(t2[:], mag[:], 1e-5)
                den = stats_pool.tile([H, HALVES * NQ], f32, tag="den")
                nc.vector.scalar_tensor_tensor(
                    out=den[:], in0=msq[:], scalar=1.0, in1=t2[:],
                    op0=mybir.AluOpType.add, op1=mybir.AluOpType.mult)
                rec = stats_pool.tile([H, HALVES * NQ], f32, tag="rec")
                nc.vector.reciprocal(rec[:], den[:])
                fac = stats_pool.tile([H, HALVES * NQ], f32, tag="fac")
                nc.vector.tensor_mul(fac[:], msq[:], rec[:])

                for hf in range(HALVES):
                    tg = str(hf)
                    outt = out_pool.tile([H, 512], f32, tag="out" + tg)
                    fap = fac[:, hf * NQ:(hf + 1) * NQ]
                    fac_bcast = bass.AP(tensor=fap.tensor, offset=fap.offset,
                                        ap=[fap.ap[0], fap.ap[1], [0, 128]])
                    nc.vector.tensor_mul(
                        outt[:].rearrange("p (q u) -> p q u", q=NQ),
                        srcs[hf].rearrange("p (q u) -> p q u", q=NQ),
                        fac_bcast)
                    # All stores ride the HWDGE rings (a 16 KB store is
                    # ~45 ns of ring time behind the queued loads). Keeping
                    # SWDGE idle makes the end-of-kernel GpSimd DRAIN --
                    # which waits for its queue -- instant.
                    eng = nc.sync if hf == 0 else nc.scalar
                    eng.dma_start(out=y[b, hf * H:(hf + 1) * H],
                                  in_=outt[:])

            for b in range(PB):
                for h in range(HALVES):
                    load_and_mm(b, h)
                if b >= LOOKAHEAD:
                    squash(b - LOOKAHEAD)
            for b in range(PB - LOOKAHEAD, PB):
                squash(b)

    nc.compile()
    return nc


_NC_CACHE = {}


def _get_nc(**kw):
    key = tuple(sorted(kw.items()))
    if key not in _NC_CACHE:
        _NC_CACHE[key] = build_bass(**kw)
    return _NC_CACHE[key]


def kernel(x, **run_kwargs):
    x = np.ascontiguousarray(np.asarray(x, dtype=np.float32))
    assert x.shape == (B, NU, IC, US), x.shape

    nc = _get_nc()
    xs = x.reshape(N_CORES, PB, HALVES, 128, F)
    w = np.zeros((128, 5, 4), dtype=np.float32)
    for m in range(4):
        w[:, m, m] = 1.0 / IC
    in_maps = [{"x": np.ascontiguousarray(xs[c]), "w": w}
               for c in range(N_CORES)]
    res = run_bass_kernel_spmd(nc, in_maps, core_ids=list(range(N_CORES)),
                               **run_kwargs)
    out = np.stack([r["y"] for r in res.results], axis=0)  # (8, PB, 8, 512)
    out = out.reshape(B, NU, US, 1)
    if run_kwargs:
        kernel.last_results = res
    return out
